# revision 40
# baseline (speedup 1.0000x reference)
"""Trainium2 Bass kernel for nn_Attention_71210557768228.

Single-layer non-causal attention with RoPE; 8 cores = (batch, head-group).

Numerics (per-stage, all validated against the fp32 reference):
  - scores: fp8e4 DoubleRow matmuls over packed [32 p, 2 u, T] per-head q/k
    (post-RoPE, one head per quadrant, explicit tile_position).
  - q/k + v projections: fp8 DoubleRow with residual splits (x = x8+xl8,
    W*64 = W8+Wl8; 64x cancels via exp scale and a 64-valued ones column).
    Term count per tile is tunable: the 4 startup tiles run 1-term (so the
    first scores only need x8+W8 loaded), k03 tq1-3 tiles run 2-term
    (x-residual only), the rest 3-term.
  - attn@v and out-projection stay bf16.

attn@v layout ("layout B"): output is [128 query-partitions, 65-moving]
(64 head dims + the WS ones column), contraction over 128 keys per
instruction.  The moving dim is 65 instead of 512, so attn@v costs
65/128 of the partition-dim-wasteful layout.  The per-(head,it) PSUM
accumulator yu is [128, 4, 65] (4 query subtiles of 128).  Softmax
normalization becomes a per-partition scalar multiply (reciprocal of
the ones column), and the [q, d] -> [d, q] layout fix for the output
projection is done by PE transposes of head PAIRS (even+odd head y
halves concatenated on the free axis so the transpose lands both
partition halves of y2 at once).

Schedule: head-outer, it-inner, A/B super-block interleave, JIT v
tiles, trickled qk fills and dribbled projection tiles as in the
original; the last block dribbles per-query-subtile so the tail is
short.
"""

import os
import sys

import numpy as np
import ml_dtypes

_REPO = "/opt/trn_rl_repo"
if _REPO not in sys.path:
    sys.path.insert(0, _REPO)

import concourse.bass as bass
import concourse.bacc as bacc
import concourse.mybir as mybir
import concourse.tile as tile
from concourse.bass import ts
from concourse.tile import TileContext

F32 = mybir.dt.float32
BF16 = mybir.dt.bfloat16
FP8 = mybir.dt.float8e4

DIM, H, D = 1024, 16, 64
B, T = 4, 2048
G = 2
HG = H // G
DV = HG * D
N_CORES = 8

SWAP16 = [(i + 16) % 32 for i in range(32)]
WS = 64.0

CT2 = DIM // 256
TT = T // 128
T4 = T // 512
JT = T // 128
NFT = 4               # 0=Q(h0-3) 1=K(h0-3) 2=Q(h4-7) 3=K(h4-7)

DR = mybir.MatmulPerfMode.DoubleRow


def _dmap():
    dm = np.zeros((32, 2), dtype=np.int64)
    for u in range(2):
        for i in range(32):
            j = u * 16 + (i % 16)
            elem = i // 16
            dm[i, u] = 2 * j + elem
    return dm


def _cos_sin_tiles(freqs):
    dm = _dmap()
    cos = np.cos(freqs)
    sin = np.sin(freqs)
    cosP = np.zeros((128, 2, T), dtype=np.float32)
    sinS = np.zeros((128, 2, T), dtype=np.float32)
    for i in range(32):
        for u in range(2):
            d = dm[i, u]
            isw = (i + 16) % 32
            d_dest = dm[isw, u]
            sign_dest = -1.0 if isw < 16 else 1.0
            for q in range(4):
                cosP[32 * q + i, u] = cos[:, d]
                sinS[32 * q + i, u] = sign_dest * sin[:, d_dest]
    return cosP.astype(ml_dtypes.bfloat16), sinS.astype(ml_dtypes.bfloat16)


# ---------------------------------------------------------------- bass build

def build_nc(pexp_bufs=24, rope_bufs=3, yu_bufs=2, norm_bufs=3, osb_bufs=5,
             s_bufs=2, startup_terms=1, k123_terms=1, ynp_bufs=2,
             debug_y2=False):
    nc = bacc.Bacc("TRN2", target_bir_lowering=False)

    x8_d = nc.dram_tensor("x8", (DIM, T), FP8, kind="ExternalInput")
    xl_d = nc.dram_tensor("xl8", (DIM, T), FP8, kind="ExternalInput")
    wqk8_d = nc.dram_tensor("wqk8", (DIM, NFT * 2 * 128), FP8, kind="ExternalInput")
    wqkl_d = nc.dram_tensor("wqkl8", (DIM, NFT * 2 * 128), FP8, kind="ExternalInput")
    wv2_d = nc.dram_tensor("wv28", (DIM, 2 * DV), FP8, kind="ExternalInput")
    wp_d = nc.dram_tensor("wpT", (DV, DIM), BF16, kind="ExternalInput")
    trig_d = nc.dram_tensor("trig", (128, 2 * 2 * T), BF16, kind="ExternalInput")
    id_d = nc.dram_tensor("ident", (128, 128), F32, kind="ExternalInput")
    out_d = nc.dram_tensor("out_part", (T, DIM), F32, kind="ExternalOutput")
    y2dbg_d = nc.dram_tensor("y2dbg", (128, DV // 128 * T), BF16,
                             kind="ExternalOutput") if debug_y2 else None

    def dsrc(dten, cols):
        return dten[:, :].rearrange("(c p) t -> p c t", p=128)[:, :, cols]

    with TileContext(nc) as tc:
        with tc.tile_pool(name="const", bufs=1) as cpool:
            x8_sb = cpool.tile([128, CT2, 2, T], FP8)
            xl_sb = cpool.tile([128, CT2, 2, T], FP8)
            wqk8_sb = cpool.tile([128, CT2, 2, NFT, 2, 128], FP8)
            wqkl_sb = cpool.tile([128, CT2, 2, NFT, 2, 128], FP8)
            wv2_sb = cpool.tile([128, CT2, 2, 2, DV], FP8)
            wv8_sb = wv2_sb[:, :, :, 0]
            wvl_sb = wv2_sb[:, :, :, 1]
            wp_sb = cpool.tile([128, DV // 128, DIM], BF16)
            trig_sb = cpool.tile([128, 2, 2, T], BF16)
            cos_sb = trig_sb[:, 0]
            sin_sb = trig_sb[:, 1]
            qk8 = cpool.tile([128, NFT, 2, T], FP8)
            v_sb = cpool.tile([128, JT, HG, D + 1], BF16)
            y2_sb = cpool.tile([128, DV // 128, T], BF16)
            ident_sb = cpool.tile([128, 128], F32)

            nc.vector.memset(v_sb[:, :, :, D], WS)

            with tc.tile_pool(name="pA", bufs=1) as apool, \
                 tc.tile_pool(name="ps", bufs=1, space="PSUM") as psum, \
                 tc.tile_pool(name="rope", bufs=rope_bufs) as rpool, \
                 tc.tile_pool(name="pexp", bufs=pexp_bufs) as pxpool, \
                 tc.tile_pool(name="norm", bufs=norm_bufs) as npool, \
                 tc.tile_pool(name="ynp", bufs=ynp_bufs) as ypool, \
                 tc.tile_pool(name="osb", bufs=osb_bufs) as opool:

                # ---- input DMAs: one ordered stream on sync (transfers
                # serialize on a single DMA track; order = arrival schedule)
                def tsrc(cols):
                    return trig_d[:, :].rearrange(
                        "p (a u t) -> p a u t", a=2, u=2)[:, :, :, cols]

                nc.sync.dma_start(
                    wqk8_sb[:, :, :, 0:2, :, :], dsrc(wqk8_d, slice(0, 512)))
                nc.sync.dma_start(x8_sb[:, :, :, 0:256],
                                  dsrc(x8_d, slice(0, 256)))
                nc.sync.dma_start(trig_sb[:, :, :, 0:256], tsrc(slice(0, 256)))
                nc.sync.dma_start(x8_sb[:, :, :, 256:512],
                                  dsrc(x8_d, slice(256, 512)))
                nc.sync.dma_start(trig_sb[:, :, :, 256:512],
                                  tsrc(slice(256, 512)))
                nc.sync.dma_start(
                    wv2_sb[:, :, :, :, :].rearrange("p c u a d -> p (c u) (a d)"),
                    wv2_d[:, :].rearrange("(c p) d -> p c d", p=128))
                nc.sync.dma_start(xl_sb[:, :, :, 0:512],
                                  dsrc(xl_d, slice(0, 512)))
                nc.sync.dma_start(x8_sb[:, :, :, 512:1024],
                                  dsrc(x8_d, slice(512, 1024)))
                nc.sync.dma_start(trig_sb[:, :, :, 512:1024],
                                  tsrc(slice(512, 1024)))
                nc.sync.dma_start(x8_sb[:, :, :, 1024:1536],
                                  dsrc(x8_d, slice(1024, 1536)))
                nc.sync.dma_start(trig_sb[:, :, :, 1024:1536],
                                  tsrc(slice(1024, 1536)))
                nc.sync.dma_start(x8_sb[:, :, :, 1536:T],
                                  dsrc(x8_d, slice(1536, T)))
                nc.sync.dma_start(trig_sb[:, :, :, 1536:T],
                                  tsrc(slice(1536, T)))
                nc.sync.dma_start(xl_sb[:, :, :, 512:1024],
                                  dsrc(xl_d, slice(512, 1024)))
                nc.sync.dma_start(
                    wqkl_sb[:, :, :, 0:2, :, :], dsrc(wqkl_d, slice(0, 512)))
                nc.sync.dma_start(ident_sb[:, :], id_d[:, :])
                nc.sync.dma_start(xl_sb[:, :, :, 1024:T],
                                  dsrc(xl_d, slice(1024, T)))
                nc.sync.dma_start(
                    wqk8_sb[:, :, :, 2:4, :, :], dsrc(wqk8_d, slice(512, 1024)))
                nc.sync.dma_start(
                    wqkl_sb[:, :, :, 2:4, :, :], dsrc(wqkl_d, slice(512, 1024)))
                nc.sync.dma_start(
                    wp_sb[:, :, :],
                    wp_d[:, :].rearrange("(c p) o -> p c o", p=128))

                vcpy = nc.vector

                def qk_tile(ft, u, tq, borrow=None, terms=3, tk0=0,
                            tkn=512, addq=None):
                    if borrow == "s":
                        pqk = psum.tile([128, 2, 512], F32, tag="s",
                                        bufs=s_bufs, name="pqkb")[:, 0, 0:tkn]
                    elif borrow == "pv":
                        pqk = psum.tile([128, 512], F32, tag="pv", bufs=1,
                                        name="pqkv")[:, 0:tkn]
                    else:
                        pqk = psum.tile([128, 512], F32, tag="pqk", bufs=1,
                                        name="pqk")[:, 0:tkn]
                    if terms == 2:
                        # W-residual: keeps xl off the startup critical path
                        tl = [(wqk8_sb, x8_sb), (wqkl_sb, x8_sb)]
                    else:
                        tl = [(wqk8_sb, x8_sb), (wqk8_sb, xl_sb),
                              (wqkl_sb, x8_sb)][:terms]
                    n = len(tl) * CT2
                    i = 0
                    tok = slice(512 * tq + tk0, 512 * tq + tk0 + tkn)
                    for wsb, xsb in tl:
                        for ct2 in range(CT2):
                            nc.tensor.matmul(
                                pqk,
                                lhsT=wsb[:, ct2, :, ft, u, :],
                                rhs=xsb[:, ct2, :, tok],
                                start=(i == 0), stop=(i == n - 1),
                                perf_mode=DR)
                            i += 1
                    tcos = rpool.tile([128, 512], BF16, tag="tcos", name="tcos")
                    tsin = rpool.tile([128, 512], BF16, tag="tsin", name="tsin")
                    tsw = rpool.tile([128, 512], BF16, tag="tsw", name="tsw")
                    tcos, tsin, tsw = tcos[:, 0:tkn], tsin[:, 0:tkn], tsw[:, 0:tkn]
                    nc.vector.tensor_mul(tcos, pqk, cos_sb[:, u, tok])
                    nc.vector.tensor_mul(tsin, pqk, sin_sb[:, u, tok])
                    nc.vector.stream_shuffle(tsw, tsin, SWAP16)
                    # final add reads only SBUF: by default on the idle
                    # Pool engine to keep DVE free for PSUM-bound work
                    (addq or nc.gpsimd).tensor_add(qk8[:, ft, u, tok],
                                                   tcos, tsw)

                def v_tile(tt):
                    pv = psum.tile([128, DV], F32, tag="pv", bufs=1, name="pv")
                    tl = [(x8_sb, wv8_sb), (x8_sb, wvl_sb), (xl_sb, wv8_sb)]
                    n = len(tl) * CT2
                    i = 0
                    for xsb, wsb in tl:
                        for ct2 in range(CT2):
                            nc.tensor.matmul(
                                pv,
                                lhsT=xsb[:, ct2, :, ts(tt, 128)],
                                rhs=wsb[:, ct2, :, :],
                                start=(i == 0), stop=(i == n - 1),
                                perf_mode=DR)
                            i += 1
                    vcpy.tensor_copy(
                        v_sb[:, tt, :, 0:D],
                        pv.rearrange("p (h d) -> p h d", h=HG))

                proj_done = []

                def proj_tile(tt, on, wide=False, tail=False):
                    k = len(proj_done)
                    if wide:
                        tg = ("pqk", "pv", "s")[k % 3]
                        if tg == "s":
                            po = psum.tile([128, 2, 512], F32, tag="s",
                                           bufs=s_bufs, name="pow")[:, 0, :]
                        else:
                            po = psum.tile([128, 512], F32, tag=tg, bufs=1,
                                           name="po")
                    else:
                        po = psum.tile([128, 512], F32,
                                       tag=("pqk" if (2 * tt + on) % 2 else "pv"),
                                       bufs=1, name="po")
                    for d4 in range(DV // 128):
                        nc.tensor.matmul(
                            po,
                            lhsT=y2_sb[:, d4, ts(tt, 128)],
                            rhs=wp_sb[:, d4, ts(on, 512)],
                            start=(d4 == 0), stop=(d4 == DV // 128 - 1))
                    ot = opool.tile([128, 512], F32, tag="ot")
                    if tail and on == 0:
                        # post-last-exp: ACT is idle, overlap the two copies
                        nc.scalar.copy(ot, po)
                    else:
                        nc.vector.tensor_copy(ot, po)
                    outq = nc.sync if tail else [nc.sync, nc.gpsimd][k % 2]
                    outq.dma_start(out_d[ts(tt, 128), ts(on, 512)], ot)
                    proj_done.append((tt, on))

                # fill work emitted inside attention streams, keyed (it, h, jc)
                FILL = {}

                def add_fill(it, h, jc, fn, *args, **kw):
                    FILL.setdefault((it, h, jc), []).append((fn, args, kw))

                # it0 block h0: k03 tq1-3 JIT (needed by jc2/4/6); adds
                # on DVE (latency-critical chain)
                for i, tq in enumerate((1, 2, 3)):
                    add_fill(0, 0, i, qk_tile, 1, 0, tq, terms=k123_terms)
                    add_fill(0, 0, i, qk_tile, 1, 1, tq, terms=k123_terms)
                # super-block order: A-it0, A-it1, B-it0, A-it2, B-it1,
                # A-it3, B-it2, B-it3  (A = h0-3, B = h4-7).
                # q03-it1 by A-it1: fills in A-it0 blocks 2-3
                add_fill(0, 2, 4, qk_tile, 0, 0, 1, terms=3)
                add_fill(0, 3, 4, qk_tile, 0, 1, 1, terms=3)
                # ft2/ft3 (q47-tq0 + k47 all tq) by B-it0: spread over
                # A-it0 blocks 2-3 and all of A-it1
                spread = [(0, 2, 5, (2, 0, 0), 3), (0, 3, 5, (2, 1, 0), 3),
                          (1, 0, 2, (3, 0, 0), 1), (1, 0, 5, (3, 1, 0), 1),
                          (1, 1, 2, (3, 0, 1), 1), (1, 1, 5, (3, 1, 1), 1),
                          (1, 2, 2, (3, 0, 2), 1), (1, 2, 5, (3, 1, 2), 1),
                          (1, 3, 2, (3, 0, 3), 1), (1, 3, 5, (3, 1, 3), 1)]
                for it_, h, jc, (ft, u, tq), terms in spread:
                    add_fill(it_, h, jc, qk_tile, ft, u, tq, terms=terms)
                # q03-it2 + q47-it1 by their supers: fills in B-it0
                add_fill(0, 4, 2, qk_tile, 0, 0, 2, terms=3)
                add_fill(0, 5, 2, qk_tile, 0, 1, 2, terms=3)
                add_fill(0, 6, 2, qk_tile, 2, 0, 1, terms=3)
                add_fill(0, 7, 2, qk_tile, 2, 1, 1, terms=3)
                # q03-it3 in A-it2; q47-it2 in B-it1; q47-it3 in B-it2
                add_fill(2, 0, 3, qk_tile, 0, 0, 3, terms=3)
                add_fill(2, 1, 3, qk_tile, 0, 1, 3, terms=3)
                add_fill(1, 4, 3, qk_tile, 2, 0, 2, terms=3)
                add_fill(1, 5, 3, qk_tile, 2, 1, 2, terms=3)
                add_fill(2, 4, 3, qk_tile, 2, 0, 3, terms=3)
                add_fill(2, 5, 3, qk_tile, 2, 1, 3, terms=3)

                # dribble schedule for the group-interleaved order:
                # it0 tokens ready after B-it0 -> dribble in A-it2 (jc3+jc6)
                # it1 tokens ready after B-it1 -> dribble in A-it3
                # it2 tokens ready after B-it2 -> dribble in B-it3
                DRIB = {}
                # (shifted one block vs the un-lagged schedule: y2 for a
                # head pair lands one block after the pair's odd head)
                for h in range(1, 4):
                    for jc in (1, 4, 7):
                        DRIB[(2, h, jc)] = 0
                        DRIB[(3, h, jc)] = 1
                for h in range(5, 7):
                    for jc in (1, 3, 5, 7):
                        DRIB[(3, h, jc)] = 2

                # y_norm pair stash: one per (it) while a head pair is in
                # flight; [q, qs, parity, d] so the pair transpose reads a
                # contiguous 128-wide free block per qs
                ynp_live = {}

                # deferred PE work (pair transposes) flushed after the next
                # block's first scores so the PE never waits on the norm DVE
                PENDING_PE = []

                def norm_step(h, it, yu):
                    """recip + per-qs scalar muls into the pair stash; on odd
                    heads defer the pair transpose + y2 copy.  Per-qs [128,1]
                    reciprocals with offset-0 scalar operands: strided
                    reciprocal reads and free-offset scalar APs misbehave on
                    hardware."""
                    key = it
                    if h % 2 == 0:
                        ynp = ypool.tile([128, 4, 2, D], F32, tag="ynp")
                        ynp_live[key] = ynp
                    else:
                        ynp = ynp_live[key]
                    for qs in range(4):
                        rcp = npool.tile([128, 1], F32, tag="rcp")
                        nc.vector.reciprocal(rcp, yu[:, qs, D:D + 1])
                        nc.vector.tensor_scalar_mul(
                            ynp[:, qs, h % 2, :], yu[:, qs, 0:D], rcp)
                    if h % 2 == 1:
                        hp = h // 2

                        def finish(ynp=ynp, hp=hp, it=it):
                            # one fresh [128,128] psum tile per transpose:
                            # sub-bank-offset transpose outputs clobber each
                            # other on hardware
                            for qs in range(4):
                                yt2 = psum.tile([128, 128], F32, tag="pv",
                                                bufs=1, name="yt2")
                                nc.tensor.transpose(
                                    yt2, ynp[:, qs, :, :].rearrange(
                                        "p a b -> p (a b)"),
                                    ident_sb[:, :])
                                nc.vector.tensor_copy(
                                    y2_sb[:, hp, ts(4 * it + qs, 128)], yt2)
                        PENDING_PE.append(finish)

                # attn@v runs ONE BLOCK BEHIND the scores/exp stream: the PE
                # tracks a single open accumulation group per PSUM bank, so
                # each query-subtile's 16-step contraction must be emitted
                # contiguously.  The pexp ring (20 bufs) holds the previous
                # block's 8 chunks while the current block streams.
                prev_blk = [None]   # {"h","it","yu","px"}

                def att_chunk(h, it, jc, cur):
                    ftq, ftk, base = 2 * (h // 4), 2 * (h // 4) + 1, 32 * (h % 4)
                    sp = psum.tile([128, 2, 512], F32, tag="s",
                                   bufs=s_bufs, name="s")
                    for u2 in range(2):
                        jt = 2 * jc + u2
                        nc.tensor.matmul(
                            sp[:, u2, :],
                            lhsT=qk8[base:base + 32, ftk, :, ts(jt, 128)],
                            rhs=qk8[base:base + 32, ftq, :, ts(it, 512)],
                            start=True, stop=True,
                            perf_mode=DR, tile_position=(base, 0))
                    pexp = pxpool.tile([128, 2, 512], BF16, tag="px",
                                       bufs=pexp_bufs, name="px")
                    nc.scalar.activation(
                        pexp, sp[:],
                        mybir.ActivationFunctionType.Exp,
                        scale=0.125 / (WS * WS))
                    cur["px"][jc] = pexp
                    if PENDING_PE:
                        while PENDING_PE:
                            PENDING_PE.pop(0)()
                    for fn, args, kw in FILL.get((it, h, jc), ()):
                        fn(*args, **kw)
                    key = (it, h, jc)
                    if key in DRIB:
                        ready = [(tt, on)
                                 for tt in range(4 * DRIB[key],
                                                 4 * DRIB[key] + 4)
                                 for on in range(DIM // 512)
                                 if (tt, on) not in proj_done]
                        if ready:
                            proj_tile(*ready[0])

                def lag_attnv(blk, qs):
                    """One query-subtile's full contiguous key contraction
                    for the previous block."""
                    h = blk["h"]
                    for jt in range(JT):
                        nc.tensor.matmul(
                            blk["yu"][:, qs, :],
                            lhsT=blk["px"][jt // 2][:, jt % 2, ts(qs, 128)],
                            rhs=v_sb[:, jt, h, :],
                            start=(jt == 0), stop=(jt == JT - 1))

                v_done = [0]

                gchunk = [0]

                def att_block(h, it, lags=("prev",), lag_jcs=(1, 3, 5, 7)):
                    # lags: blocks (dicts) whose attn@v+norm run during this
                    # block; "prev" = prev_blk.  4 sweeps per lagged block,
                    # spread over lag_jcs (all 8 jcs when 2 blocks lag).
                    cur = {"h": h, "it": it, "yu": None, "px": {}}
                    todo = []
                    for lg in lags:
                        blk = prev_blk[0] if lg == "prev" else lg
                        if blk is not None:
                            todo.append(blk)
                    sweeps = [(blk, qs) for blk in todo for qs in range(4)]
                    jcs = list(lag_jcs) if len(sweeps) <= 4 else list(range(8))
                    for jc in range(JT // 2):
                        # v tiles: ~2 of every 3 chunks until all 16 are out
                        if v_done[0] < JT and gchunk[0] < 24 \
                                and gchunk[0] % 3 != 2:
                            v_tile(v_done[0])
                            v_done[0] += 1
                        gchunk[0] += 1
                        att_chunk(h, it, jc, cur)
                        while sweeps and jcs and jc == jcs[0]:
                            blk, qs = sweeps.pop(0)
                            if blk["yu"] is None:
                                blk["yu"] = psum.tile(
                                    [128, 4, D + 1], F32, tag="yu",
                                    bufs=yu_bufs, name="yu")
                            lag_attnv(blk, qs)
                            jcs.pop(0)
                    for blk in todo:
                        norm_step(blk["h"], blk["it"], blk["yu"])
                    prev_blk[0] = cur

                def att_block_tail(h, it):
                    """Last block (h odd), query-subtile-major chunks; also
                    consumes the previous block's (pair partner's) lagged
                    attn@v per qs, so each qs finishes both heads and its
                    projection while later exps still stream."""
                    while PENDING_PE:
                        PENDING_PE.pop(0)()
                    ftq, ftk = 2 * (h // 4), 2 * (h // 4) + 1
                    base = 32 * (h % 4)
                    yu = psum.tile([128, 4, D + 1], F32, tag="yu",
                                   bufs=yu_bufs, name="yu")
                    pb = prev_blk[0]
                    hp = h // 2
                    ynp = ypool.tile([128, 4, 2, D], F32, tag="ynp")
                    for qs in range(4):
                        for half in range(2):
                            sp = psum.tile([128, 8, 128], F32, tag="s",
                                           bufs=s_bufs, name="st")
                            for j8 in range(8):
                                jt = 8 * half + j8
                                nc.tensor.matmul(
                                    sp[:, j8, :],
                                    lhsT=qk8[base:base + 32, ftk, :,
                                             ts(jt, 128)],
                                    rhs=qk8[base:base + 32, ftq, :,
                                            it * 512 + 128 * qs:
                                            it * 512 + 128 * (qs + 1)],
                                    start=True, stop=True,
                                    perf_mode=DR, tile_position=(base, 0))
                            pexp = pxpool.tile([128, 8, 128], BF16, tag="pxt",
                                               bufs=2, name="pxt")
                            nc.scalar.activation(
                                pexp, sp[:],
                                mybir.ActivationFunctionType.Exp,
                                scale=0.125 / (WS * WS))
                            if half == 0:
                                # partner head's lagged attn@v for this qs
                                if pb["yu"] is None:
                                    pb["yu"] = psum.tile(
                                        [128, 4, D + 1], F32, tag="yu",
                                        bufs=yu_bufs, name="yu")
                                lag_attnv(pb, qs)
                                rcpp = npool.tile([128, 1], F32, tag="rcp",
                                                  name="rcpp")
                                nc.vector.reciprocal(
                                    rcpp, pb["yu"][:, qs, D:D + 1])
                                nc.vector.tensor_scalar_mul(
                                    ynp[:, qs, 0, :],
                                    pb["yu"][:, qs, 0:D], rcpp)
                            for j8 in range(8):
                                jt = 8 * half + j8
                                nc.tensor.matmul(
                                    yu[:, qs, :],
                                    lhsT=pexp[:, j8, :],
                                    rhs=v_sb[:, jt, h, :],
                                    start=(jt == 0), stop=(jt == JT - 1))
                        rcp = npool.tile([128, 1], F32, tag="rcp")
                        nc.vector.reciprocal(rcp, yu[:, qs, D:D + 1])
                        nc.vector.tensor_scalar_mul(
                            ynp[:, qs, 1, :], yu[:, qs, 0:D], rcp)
                        yt2 = psum.tile([128, 128], F32, tag="pv",
                                        bufs=1, name="yt2t")
                        nc.tensor.transpose(
                            yt2, ynp[:, qs, :, :].rearrange("p a b -> p (a b)"),
                            ident_sb[:, :])
                        tt = 4 * it + qs
                        nc.vector.tensor_copy(
                            y2_sb[:, hp, ts(tt, 128)], yt2)
                        for on in range(DIM // 512):
                            if (tt, on) not in proj_done:
                                proj_tile(tt, on, tail=(qs == 3))

                # ---- phase A: minimal startup ----
                # all startup tiles split into 256-token halves: the rope
                # chains start as soon as the first x8/trig half lands
                qk_tile(0, 0, 0, borrow="s", terms=startup_terms,
                        tk0=0, tkn=256)
                qk_tile(0, 1, 0, borrow="s", terms=startup_terms,
                        tk0=0, tkn=256)
                qk_tile(0, 0, 0, borrow="s", terms=startup_terms,
                        tk0=256, tkn=256)
                qk_tile(0, 1, 0, borrow="s", terms=startup_terms,
                        tk0=256, tkn=256)
                qk_tile(1, 0, 0, borrow="pv", terms=startup_terms,
                        tk0=0, tkn=256)
                qk_tile(1, 1, 0, borrow="pqk", terms=startup_terms,
                        tk0=0, tkn=256)
                qk_tile(1, 0, 0, borrow="pv", terms=startup_terms,
                        tk0=256, tkn=256)
                qk_tile(1, 1, 0, borrow="pqk", terms=startup_terms,
                        tk0=256, tkn=256)
                for _ in range(3):
                    v_tile(v_done[0]); v_done[0] += 1

                # A-it0 with a lag-2 ramp: blocks 0-1 lag nothing (the
                # v tiles own the PE), block2 lags blk0 late, block3 lags
                # blk1, block4 catches up with blk2+blk3, then steady lag-1
                blks = {}

                def ab(h, it, **kw):
                    att_block(h, it, **kw)
                    blks[(h, it)] = prev_blk[0]

                ab(0, 0, lags=())
                ab(1, 0, lags=())
                ab(2, 0, lags=(blks[(0, 0)],), lag_jcs=(4, 5, 6, 7))
                ab(3, 0, lags=(blks[(1, 0)],))
                ab(0, 1, lags=(blks[(2, 0)], blks[(3, 0)]))
                for h in range(1, 4):
                    ab(h, 1)
                for h in range(4, 8):
                    ab(h, 0)
                for h in range(4):
                    ab(h, 2)
                for h in range(4, 8):
                    ab(h, 1)
                for h in range(4):
                    ab(h, 3)
                for h in range(4, 8):
                    ab(h, 2)
                # B-it3: h6 before h7; h7 last with the per-qs tail dribble
                ab(4, 3)
                ab(5, 3)
                ab(6, 3)
                att_block_tail(7, 3)

                while PENDING_PE:
                    PENDING_PE.pop(0)()

                if debug_y2:
                    nc.sync.dma_start(
                        y2dbg_d[:, :],
                        y2_sb.rearrange("p a t -> p (a t)"))

                for tt in range(TT):
                    for on in range(DIM // 512):
                        if (tt, on) not in proj_done:
                            proj_tile(tt, on, wide=True)

    nc.finalize()
    return nc


_NC_CACHE = None


def _get_nc():
    global _NC_CACHE
    if _NC_CACHE is None:
        _NC_CACHE = build_nc()
    return _NC_CACHE


# ---------------------------------------------------------------- entry point

def kernel(x, freqs, W_qkv, W_proj, b_proj, _trace=False):
    x = np.asarray(x, dtype=np.float32)
    freqs = np.asarray(freqs, dtype=np.float32)
    W_qkv = np.asarray(W_qkv, dtype=np.float32)
    W_proj = np.asarray(W_proj, dtype=np.float32)
    b_proj = np.asarray(b_proj, dtype=np.float32)

    dm = _dmap()
    cosP, sinS = _cos_sin_tiles(freqs)

    def split8(a):
        hi = a.astype(ml_dtypes.float8_e4m3fn)
        lo = (a - hi.astype(np.float32)).astype(ml_dtypes.float8_e4m3fn)
        return hi, lo

    wqk8T, wqklT, wv8T, wvlT, wpT = {}, {}, {}, {}, {}
    for g in range(G):
        cols = []
        for ft in range(NFT):
            qk = ft % 2
            for u in range(2):
                for hq in range(4):
                    h = g * HG + 4 * (ft // 2) + hq
                    base = qk * DIM + h * D
                    cols.append(W_qkv[base + dm[:, u]])
        wqkT_full = np.ascontiguousarray(
            np.concatenate(cols, axis=0).T).astype(np.float32) * WS
        wqk8T[g], wqklT[g] = split8(wqkT_full)
        wvT_full = np.ascontiguousarray(
            W_qkv[2 * DIM + g * DV: 2 * DIM + (g + 1) * DV].T
        ).astype(np.float32) * WS
        wv8T[g], wvlT[g] = split8(wvT_full)
        wpT[g] = np.ascontiguousarray(
            W_proj[:, g * DV:(g + 1) * DV].T).astype(ml_dtypes.bfloat16)

    x8_b, xl_b = {}, {}
    for b in range(B):
        x8_b[b], xl_b[b] = split8(np.ascontiguousarray(x[b].T))

    trig = np.concatenate(
        [cosP.reshape(128, 2 * T), sinS.reshape(128, 2 * T)],
        axis=1)  # [128, 2*2*T] bf16: cos block then sin block
    wv2 = {g: np.concatenate([wv8T[g], wvlT[g]], axis=1) for g in range(G)}
    ident = np.eye(128, dtype=np.float32)
    in_maps = []
    for core in range(N_CORES):
        b, g = core // G, core % G
        in_maps.append({
            "x8": x8_b[b],
            "xl8": xl_b[b],
            "wqk8": wqk8T[g],
            "wqkl8": wqklT[g],
            "wv28": wv2[g],
            "wpT": wpT[g],
            "trig": trig,
            "ident": ident,
        })

    from concourse import bass_utils

    nc = _get_nc()
    res = bass_utils.run_bass_kernel_spmd(
        nc, in_maps, core_ids=list(range(N_CORES)), trace=_trace)

    out = np.zeros((B, T, DIM), dtype=np.float32)
    for core in range(N_CORES):
        b = core // G
        out[b] += res.results[core]["out_part"]
    out += b_proj
    if _trace:
        return out, res
    return out


# revision 44
# speedup vs baseline: 1.0025x; 1.0025x over previous
"""Trainium2 Bass kernel for nn_Attention_71210557768228.

Single-layer non-causal attention with RoPE; 8 cores = (batch, head-group).

Numerics (per-stage, all validated against the fp32 reference):
  - scores: fp8e4 DoubleRow matmuls over packed [32 p, 2 u, T] per-head q/k
    (post-RoPE, one head per quadrant, explicit tile_position).
  - q/k + v projections: fp8 DoubleRow with residual splits (x = x8+xl8,
    W*64 = W8+Wl8; 64x cancels via exp scale and a 64-valued ones column).
    Term count per tile is tunable: the 4 startup tiles run 1-term (so the
    first scores only need x8+W8 loaded), k03 tq1-3 tiles run 2-term
    (x-residual only), the rest 3-term.
  - attn@v and out-projection stay bf16.

attn@v layout ("layout B"): output is [128 query-partitions, 65-moving]
(64 head dims + the WS ones column), contraction over 128 keys per
instruction.  The moving dim is 65 instead of 512, so attn@v costs
65/128 of the partition-dim-wasteful layout.  The per-(head,it) PSUM
accumulator yu is [128, 4, 65] (4 query subtiles of 128).  Softmax
normalization becomes a per-partition scalar multiply (reciprocal of
the ones column), and the [q, d] -> [d, q] layout fix for the output
projection is done by PE transposes of head PAIRS (even+odd head y
halves concatenated on the free axis so the transpose lands both
partition halves of y2 at once).

Schedule: head-outer, it-inner, A/B super-block interleave, JIT v
tiles, trickled qk fills and dribbled projection tiles as in the
original; the last block dribbles per-query-subtile so the tail is
short.
"""

import os
import sys

import numpy as np
import ml_dtypes

_REPO = "/opt/trn_rl_repo"
if _REPO not in sys.path:
    sys.path.insert(0, _REPO)

import concourse.bass as bass
import concourse.bacc as bacc
import concourse.mybir as mybir
import concourse.tile as tile
from concourse.bass import ts
from concourse.tile import TileContext

F32 = mybir.dt.float32
BF16 = mybir.dt.bfloat16
FP8 = mybir.dt.float8e4

DIM, H, D = 1024, 16, 64
B, T = 4, 2048
G = 2
HG = H // G
DV = HG * D
N_CORES = 8

SWAP16 = [(i + 16) % 32 for i in range(32)]
WS = 64.0

CT2 = DIM // 256
TT = T // 128
T4 = T // 512
JT = T // 128
NFT = 4               # 0=Q(h0-3) 1=K(h0-3) 2=Q(h4-7) 3=K(h4-7)

DR = mybir.MatmulPerfMode.DoubleRow


def _dmap():
    dm = np.zeros((32, 2), dtype=np.int64)
    for u in range(2):
        for i in range(32):
            j = u * 16 + (i % 16)
            elem = i // 16
            dm[i, u] = 2 * j + elem
    return dm


def _cos_sin_tiles(freqs):
    dm = _dmap()
    cos = np.cos(freqs)
    sin = np.sin(freqs)
    cosP = np.zeros((128, 2, T), dtype=np.float32)
    sinS = np.zeros((128, 2, T), dtype=np.float32)
    for i in range(32):
        for u in range(2):
            d = dm[i, u]
            isw = (i + 16) % 32
            d_dest = dm[isw, u]
            sign_dest = -1.0 if isw < 16 else 1.0
            for q in range(4):
                cosP[32 * q + i, u] = cos[:, d]
                sinS[32 * q + i, u] = sign_dest * sin[:, d_dest]
    return cosP.astype(ml_dtypes.bfloat16), sinS.astype(ml_dtypes.bfloat16)


# ---------------------------------------------------------------- bass build

def build_nc(pexp_bufs=24, rope_bufs=3, yu_bufs=2, norm_bufs=3, osb_bufs=5,
             s_bufs=2, startup_terms=1, k123_terms=1, ynp_bufs=2,
             debug_y2=False):
    nc = bacc.Bacc("TRN2", target_bir_lowering=False)

    x8_d = nc.dram_tensor("x8", (DIM, T), FP8, kind="ExternalInput")
    xl_d = nc.dram_tensor("xl8", (DIM, T), FP8, kind="ExternalInput")
    wqk8_d = nc.dram_tensor("wqk8", (DIM, NFT * 2 * 128), FP8, kind="ExternalInput")
    wqkl_d = nc.dram_tensor("wqkl8", (DIM, NFT * 2 * 128), FP8, kind="ExternalInput")
    wv2_d = nc.dram_tensor("wv28", (DIM, 2 * DV), FP8, kind="ExternalInput")
    wp_d = nc.dram_tensor("wpT", (DV, DIM), BF16, kind="ExternalInput")
    trig_d = nc.dram_tensor("trig", (128, 2 * 2 * T), BF16, kind="ExternalInput")
    id_d = nc.dram_tensor("ident", (128, 128), F32, kind="ExternalInput")
    out_d = nc.dram_tensor("out_part", (T, DIM), F32, kind="ExternalOutput")
    y2dbg_d = nc.dram_tensor("y2dbg", (128, DV // 128 * T), BF16,
                             kind="ExternalOutput") if debug_y2 else None

    def dsrc(dten, cols):
        return dten[:, :].rearrange("(c p) t -> p c t", p=128)[:, :, cols]

    with TileContext(nc) as tc:
        with tc.tile_pool(name="const", bufs=1) as cpool:
            x8_sb = cpool.tile([128, CT2, 2, T], FP8)
            xl_sb = cpool.tile([128, CT2, 2, T], FP8)
            wqk8_sb = cpool.tile([128, CT2, 2, NFT, 2, 128], FP8)
            wqkl_sb = cpool.tile([128, CT2, 2, NFT, 2, 128], FP8)
            wv2_sb = cpool.tile([128, CT2, 2, 2, DV], FP8)
            wv8_sb = wv2_sb[:, :, :, 0]
            wvl_sb = wv2_sb[:, :, :, 1]
            wp_sb = cpool.tile([128, DV // 128, DIM], BF16)
            trig_sb = cpool.tile([128, 2, 2, T], BF16)
            cos_sb = trig_sb[:, 0]
            sin_sb = trig_sb[:, 1]
            qk8 = cpool.tile([128, NFT, 2, T], FP8)
            v_sb = cpool.tile([128, JT, HG, D + 1], BF16)
            y2_sb = cpool.tile([128, DV // 128, T], BF16)
            ident_sb = cpool.tile([128, 128], F32)

            nc.vector.memset(v_sb[:, :, :, D], WS)

            with tc.tile_pool(name="pA", bufs=1) as apool, \
                 tc.tile_pool(name="ps", bufs=1, space="PSUM") as psum, \
                 tc.tile_pool(name="rope", bufs=rope_bufs) as rpool, \
                 tc.tile_pool(name="pexp", bufs=pexp_bufs) as pxpool, \
                 tc.tile_pool(name="norm", bufs=norm_bufs) as npool, \
                 tc.tile_pool(name="ynp", bufs=ynp_bufs) as ypool, \
                 tc.tile_pool(name="osb", bufs=osb_bufs) as opool:

                # ---- input DMAs: one ordered stream on sync (transfers
                # serialize on a single DMA track; order = arrival schedule)
                def tsrc(cols):
                    return trig_d[:, :].rearrange(
                        "p (a u t) -> p a u t", a=2, u=2)[:, :, :, cols]

                nc.sync.dma_start(
                    wqk8_sb[:, :, :, 0:2, :, :], dsrc(wqk8_d, slice(0, 512)))
                nc.sync.dma_start(x8_sb[:, :, :, 0:256],
                                  dsrc(x8_d, slice(0, 256)))
                nc.sync.dma_start(trig_sb[:, :, :, 0:256], tsrc(slice(0, 256)))
                nc.sync.dma_start(x8_sb[:, :, :, 256:512],
                                  dsrc(x8_d, slice(256, 512)))
                nc.sync.dma_start(trig_sb[:, :, :, 256:512],
                                  tsrc(slice(256, 512)))
                nc.sync.dma_start(
                    wv2_sb[:, :, :, :, :].rearrange("p c u a d -> p (c u) (a d)"),
                    wv2_d[:, :].rearrange("(c p) d -> p c d", p=128))
                nc.sync.dma_start(xl_sb[:, :, :, 0:512],
                                  dsrc(xl_d, slice(0, 512)))
                nc.sync.dma_start(x8_sb[:, :, :, 512:1024],
                                  dsrc(x8_d, slice(512, 1024)))
                nc.sync.dma_start(trig_sb[:, :, :, 512:1024],
                                  tsrc(slice(512, 1024)))
                nc.sync.dma_start(x8_sb[:, :, :, 1024:1536],
                                  dsrc(x8_d, slice(1024, 1536)))
                nc.sync.dma_start(trig_sb[:, :, :, 1024:1536],
                                  tsrc(slice(1024, 1536)))
                nc.sync.dma_start(x8_sb[:, :, :, 1536:T],
                                  dsrc(x8_d, slice(1536, T)))
                nc.sync.dma_start(trig_sb[:, :, :, 1536:T],
                                  tsrc(slice(1536, T)))
                nc.sync.dma_start(xl_sb[:, :, :, 512:1024],
                                  dsrc(xl_d, slice(512, 1024)))
                nc.sync.dma_start(
                    wqkl_sb[:, :, :, 0:2, :, :], dsrc(wqkl_d, slice(0, 512)))
                nc.sync.dma_start(ident_sb[:, :], id_d[:, :])
                nc.sync.dma_start(xl_sb[:, :, :, 1024:T],
                                  dsrc(xl_d, slice(1024, T)))
                nc.sync.dma_start(
                    wqk8_sb[:, :, :, 2:4, :, :], dsrc(wqk8_d, slice(512, 1024)))
                nc.sync.dma_start(
                    wqkl_sb[:, :, :, 2:4, :, :], dsrc(wqkl_d, slice(512, 1024)))
                nc.sync.dma_start(
                    wp_sb[:, :, :],
                    wp_d[:, :].rearrange("(c p) o -> p c o", p=128))

                vcpy = nc.vector

                def qk_tile(ft, u, tq, borrow=None, terms=3, tk0=0,
                            tkn=512, addq=None):
                    if borrow == "s":
                        pqk = psum.tile([128, 2, 512], F32, tag="s",
                                        bufs=s_bufs, name="pqkb")[:, 0, 0:tkn]
                    elif borrow == "pv":
                        pqk = psum.tile([128, 512], F32, tag="pv", bufs=1,
                                        name="pqkv")[:, 0:tkn]
                    else:
                        pqk = psum.tile([128, 512], F32, tag="pqk", bufs=1,
                                        name="pqk")[:, 0:tkn]
                    if terms == 2:
                        # W-residual: keeps xl off the startup critical path
                        tl = [(wqk8_sb, x8_sb), (wqkl_sb, x8_sb)]
                    else:
                        tl = [(wqk8_sb, x8_sb), (wqk8_sb, xl_sb),
                              (wqkl_sb, x8_sb)][:terms]
                    n = len(tl) * CT2
                    i = 0
                    tok = slice(512 * tq + tk0, 512 * tq + tk0 + tkn)
                    for wsb, xsb in tl:
                        for ct2 in range(CT2):
                            nc.tensor.matmul(
                                pqk,
                                lhsT=wsb[:, ct2, :, ft, u, :],
                                rhs=xsb[:, ct2, :, tok],
                                start=(i == 0), stop=(i == n - 1),
                                perf_mode=DR)
                            i += 1
                    tcos = rpool.tile([128, 512], BF16, tag="tcos", name="tcos")
                    tsin = rpool.tile([128, 512], BF16, tag="tsin", name="tsin")
                    tsw = rpool.tile([128, 512], BF16, tag="tsw", name="tsw")
                    tcos, tsin, tsw = tcos[:, 0:tkn], tsin[:, 0:tkn], tsw[:, 0:tkn]
                    nc.vector.tensor_mul(tcos, pqk, cos_sb[:, u, tok])
                    nc.vector.tensor_mul(tsin, pqk, sin_sb[:, u, tok])
                    nc.vector.stream_shuffle(tsw, tsin, SWAP16)
                    # final add reads only SBUF: by default on the idle
                    # Pool engine to keep DVE free for PSUM-bound work
                    (addq or nc.gpsimd).tensor_add(qk8[:, ft, u, tok],
                                                   tcos, tsw)

                def v_tile(tt):
                    pv = psum.tile([128, DV], F32, tag="pv", bufs=1, name="pv")
                    tl = [(x8_sb, wv8_sb), (x8_sb, wvl_sb), (xl_sb, wv8_sb)]
                    n = len(tl) * CT2
                    i = 0
                    for xsb, wsb in tl:
                        for ct2 in range(CT2):
                            nc.tensor.matmul(
                                pv,
                                lhsT=xsb[:, ct2, :, ts(tt, 128)],
                                rhs=wsb[:, ct2, :, :],
                                start=(i == 0), stop=(i == n - 1),
                                perf_mode=DR)
                            i += 1
                    vcpy.tensor_copy(
                        v_sb[:, tt, :, 0:D],
                        pv.rearrange("p (h d) -> p h d", h=HG))

                proj_done = []

                def proj_tile(tt, on, wide=False, tail=False):
                    k = len(proj_done)
                    if wide:
                        tg = ("pqk", "pv", "s")[k % 3]
                        if tg == "s":
                            po = psum.tile([128, 2, 512], F32, tag="s",
                                           bufs=s_bufs, name="pow")[:, 0, :]
                        else:
                            po = psum.tile([128, 512], F32, tag=tg, bufs=1,
                                           name="po")
                    else:
                        po = psum.tile([128, 512], F32,
                                       tag=("pqk" if (2 * tt + on) % 2 else "pv"),
                                       bufs=1, name="po")
                    for d4 in range(DV // 128):
                        nc.tensor.matmul(
                            po,
                            lhsT=y2_sb[:, d4, ts(tt, 128)],
                            rhs=wp_sb[:, d4, ts(on, 512)],
                            start=(d4 == 0), stop=(d4 == DV // 128 - 1))
                    ot = opool.tile([128, 512], F32, tag="ot")
                    if tail and on == 0:
                        # post-last-exp: ACT is idle, overlap the two copies
                        nc.scalar.copy(ot, po)
                    else:
                        nc.vector.tensor_copy(ot, po)
                    outq = nc.sync if tail else [nc.sync, nc.gpsimd][k % 2]
                    outq.dma_start(out_d[ts(tt, 128), ts(on, 512)], ot)
                    proj_done.append((tt, on))

                # fill work emitted inside attention streams, keyed (it, h, jc)
                FILL = {}

                def add_fill(it, h, jc, fn, *args, **kw):
                    FILL.setdefault((it, h, jc), []).append((fn, args, kw))

                # it0 block h0: k03 tq1-3 JIT (needed by jc2/4/6); adds
                # on DVE (latency-critical chain)
                for i, tq in enumerate((1, 2, 3)):
                    add_fill(0, 0, i, qk_tile, 1, 0, tq, terms=k123_terms)
                    add_fill(0, 0, i, qk_tile, 1, 1, tq, terms=k123_terms)
                # super-block order: A-it0, A-it1, B-it0, A-it2, B-it1,
                # A-it3, B-it2, B-it3  (A = h0-3, B = h4-7).
                # q03-it1 by A-it1: fills in A-it0 blocks 2-3
                add_fill(0, 2, 4, qk_tile, 0, 0, 1, terms=3)
                add_fill(0, 3, 4, qk_tile, 0, 1, 1, terms=3)
                # ft2/ft3 (q47-tq0 + k47 all tq) by B-it0: spread over
                # A-it0 blocks 2-3 and all of A-it1
                spread = [(0, 2, 5, (2, 0, 0), 3), (0, 3, 5, (2, 1, 0), 3),
                          (1, 0, 2, (3, 0, 0), 1), (1, 0, 5, (3, 1, 0), 1),
                          (1, 1, 2, (3, 0, 1), 1), (1, 1, 5, (3, 1, 1), 1),
                          (1, 2, 2, (3, 0, 2), 1), (1, 2, 5, (3, 1, 2), 1),
                          (1, 3, 2, (3, 0, 3), 1), (1, 3, 5, (3, 1, 3), 1)]
                for it_, h, jc, (ft, u, tq), terms in spread:
                    add_fill(it_, h, jc, qk_tile, ft, u, tq, terms=terms)
                # q03-it2 + q47-it1 by their supers: fills in B-it0
                add_fill(0, 4, 2, qk_tile, 0, 0, 2, terms=3)
                add_fill(0, 5, 2, qk_tile, 0, 1, 2, terms=3)
                add_fill(0, 6, 2, qk_tile, 2, 0, 1, terms=3)
                add_fill(0, 7, 2, qk_tile, 2, 1, 1, terms=3)
                # q03-it3 in A-it2; q47-it2 in B-it1; q47-it3 in B-it2
                add_fill(2, 0, 3, qk_tile, 0, 0, 3, terms=3)
                add_fill(2, 1, 3, qk_tile, 0, 1, 3, terms=3)
                add_fill(1, 4, 3, qk_tile, 2, 0, 2, terms=3)
                add_fill(1, 5, 3, qk_tile, 2, 1, 2, terms=3)
                add_fill(2, 4, 3, qk_tile, 2, 0, 3, terms=3)
                add_fill(2, 5, 3, qk_tile, 2, 1, 3, terms=3)

                # dribble schedule for the group-interleaved order:
                # it0 tokens ready after B-it0 -> dribble in A-it2 (jc3+jc6)
                # it1 tokens ready after B-it1 -> dribble in A-it3
                # it2 tokens ready after B-it2 -> dribble in B-it3
                DRIB = {}
                # (shifted one block vs the un-lagged schedule: y2 for a
                # head pair lands one block after the pair's odd head)
                for h in range(1, 4):
                    for jc in (1, 4, 7):
                        DRIB[(2, h, jc)] = 0
                        DRIB[(3, h, jc)] = 1
                for h in range(5, 7):
                    for jc in (1, 3, 5, 7):
                        DRIB[(3, h, jc)] = 2

                # y_norm pair stash: one per (it) while a head pair is in
                # flight; [q, qs, parity, d] so the pair transpose reads a
                # contiguous 128-wide free block per qs
                ynp_live = {}

                # deferred PE work (pair transposes) flushed after the next
                # block's first scores so the PE never waits on the norm DVE
                PENDING_PE = []

                def norm_step(h, it, yu):
                    """recip + per-qs scalar muls into the pair stash; on odd
                    heads defer the pair transpose + y2 copy.  Per-qs [128,1]
                    reciprocals with offset-0 scalar operands: strided
                    reciprocal reads and free-offset scalar APs misbehave on
                    hardware."""
                    key = it
                    if h % 2 == 0:
                        ynp = ypool.tile([128, 4, 2, D], F32, tag="ynp")
                        ynp_live[key] = ynp
                    else:
                        ynp = ynp_live[key]
                    for qs in range(4):
                        rcp = npool.tile([128, 1], F32, tag="rcp")
                        nc.vector.reciprocal(rcp, yu[:, qs, D:D + 1])
                        nc.vector.tensor_scalar_mul(
                            ynp[:, qs, h % 2, :], yu[:, qs, 0:D], rcp)
                    if h % 2 == 1:
                        hp = h // 2

                        def finish(ynp=ynp, hp=hp, it=it):
                            # one fresh [128,128] psum tile per transpose:
                            # sub-bank-offset transpose outputs clobber each
                            # other on hardware
                            for qs in range(4):
                                yt2 = psum.tile([128, 128], F32, tag="pv",
                                                bufs=1, name="yt2")
                                nc.tensor.transpose(
                                    yt2, ynp[:, qs, :, :].rearrange(
                                        "p a b -> p (a b)"),
                                    ident_sb[:, :])
                                nc.vector.tensor_copy(
                                    y2_sb[:, hp, ts(4 * it + qs, 128)], yt2)
                        PENDING_PE.append(finish)

                # attn@v runs ONE BLOCK BEHIND the scores/exp stream: the PE
                # tracks a single open accumulation group per PSUM bank, so
                # each query-subtile's 16-step contraction must be emitted
                # contiguously.  The pexp ring (20 bufs) holds the previous
                # block's 8 chunks while the current block streams.
                prev_blk = [None]   # {"h","it","yu","px"}

                def att_chunk(h, it, jc, cur):
                    ftq, ftk, base = 2 * (h // 4), 2 * (h // 4) + 1, 32 * (h % 4)
                    sp = psum.tile([128, 2, 512], F32, tag="s",
                                   bufs=s_bufs, name="s")
                    for u2 in range(2):
                        jt = 2 * jc + u2
                        nc.tensor.matmul(
                            sp[:, u2, :],
                            lhsT=qk8[base:base + 32, ftk, :, ts(jt, 128)],
                            rhs=qk8[base:base + 32, ftq, :, ts(it, 512)],
                            start=True, stop=True,
                            perf_mode=DR, tile_position=(base, 0))
                    pexp = pxpool.tile([128, 2, 512], BF16, tag="px",
                                       bufs=pexp_bufs, name="px")
                    nc.scalar.activation(
                        pexp, sp[:],
                        mybir.ActivationFunctionType.Exp,
                        scale=0.125 / (WS * WS))
                    cur["px"][jc] = pexp
                    if PENDING_PE:
                        while PENDING_PE:
                            PENDING_PE.pop(0)()
                    for fn, args, kw in FILL.get((it, h, jc), ()):
                        fn(*args, **kw)
                    key = (it, h, jc)
                    if key in DRIB:
                        ready = [(tt, on)
                                 for tt in range(4 * DRIB[key],
                                                 4 * DRIB[key] + 4)
                                 for on in range(DIM // 512)
                                 if (tt, on) not in proj_done]
                        if ready:
                            proj_tile(*ready[0])

                def lag_attnv(blk, qs):
                    """One query-subtile's full contiguous key contraction
                    for the previous block."""
                    h = blk["h"]
                    for jt in range(JT):
                        px = blk["px"].get(jt // 2)
                        if px is None:
                            # block0's chunk 0 was split into query halves
                            px = blk["px"][(0, qs // 2)]
                            sl = ts(qs % 2, 128)
                        else:
                            sl = ts(qs, 128)
                        nc.tensor.matmul(
                            blk["yu"][:, qs, :],
                            lhsT=px[:, jt % 2, sl],
                            rhs=v_sb[:, jt, h, :],
                            start=(jt == 0), stop=(jt == JT - 1))

                v_done = [0]

                gchunk = [0]

                def att_block(h, it, lags=("prev",), lag_jcs=(1, 3, 5, 7)):
                    # lags: blocks (dicts) whose attn@v+norm run during this
                    # block; "prev" = prev_blk.  4 sweeps per lagged block,
                    # spread over lag_jcs (all 8 jcs when 2 blocks lag).
                    cur = {"h": h, "it": it, "yu": None, "px": {}}
                    todo = []
                    for lg in lags:
                        blk = prev_blk[0] if lg == "prev" else lg
                        if blk is not None:
                            todo.append(blk)
                    sweeps = [(blk, qs) for blk in todo for qs in range(4)]
                    jcs = list(lag_jcs) if len(sweeps) <= 4 else list(range(8))
                    first_jc = 0
                    if h == 0 and it == 0:
                        # chunk 0 split into two 256-query minis: the first
                        # exp only waits on half the startup q rope chains
                        first_jc = 1
                        gchunk[0] += 2
                        for qh in range(2):
                            sp = psum.tile([128, 2, 512], F32, tag="s",
                                           bufs=s_bufs, name="s")
                            for u2 in range(2):
                                nc.tensor.matmul(
                                    sp[:, u2, 0:256],
                                    lhsT=qk8[0:32, 1, :, ts(u2, 128)],
                                    rhs=qk8[0:32, 0, :,
                                            256 * qh:256 * (qh + 1)],
                                    start=True, stop=True,
                                    perf_mode=DR, tile_position=(0, 0))
                            pexp = pxpool.tile([128, 2, 512], BF16, tag="px",
                                               bufs=pexp_bufs, name="px")
                            nc.scalar.activation(
                                pexp[:, :, 0:256], sp[:, :, 0:256],
                                mybir.ActivationFunctionType.Exp,
                                scale=0.125 / (WS * WS))
                            cur["px"][(0, qh)] = pexp
                        for fn, args, kw in FILL.get((it, h, 0), ()):
                            fn(*args, **kw)
                    for jc in range(first_jc, JT // 2):
                        # v tiles: ~2 of every 3 chunks until all 16 are out
                        if v_done[0] < JT and gchunk[0] < 24 \
                                and gchunk[0] % 3 != 2:
                            v_tile(v_done[0])
                            v_done[0] += 1
                        gchunk[0] += 1
                        att_chunk(h, it, jc, cur)
                        while sweeps and jcs and jc == jcs[0]:
                            blk, qs = sweeps.pop(0)
                            if blk["yu"] is None:
                                blk["yu"] = psum.tile(
                                    [128, 4, D + 1], F32, tag="yu",
                                    bufs=yu_bufs, name="yu")
                            lag_attnv(blk, qs)
                            jcs.pop(0)
                    for blk in todo:
                        norm_step(blk["h"], blk["it"], blk["yu"])
                    prev_blk[0] = cur

                def att_block_tail(h, it):
                    """Last block (h odd), query-subtile-major chunks; also
                    consumes the previous block's (pair partner's) lagged
                    attn@v per qs, so each qs finishes both heads and its
                    projection while later exps still stream."""
                    while PENDING_PE:
                        PENDING_PE.pop(0)()
                    ftq, ftk = 2 * (h // 4), 2 * (h // 4) + 1
                    base = 32 * (h % 4)
                    yu = psum.tile([128, 4, D + 1], F32, tag="yu",
                                   bufs=yu_bufs, name="yu")
                    pb = prev_blk[0]
                    hp = h // 2
                    ynp = ypool.tile([128, 4, 2, D], F32, tag="ynp")
                    for qs in range(4):
                        for half in range(2):
                            sp = psum.tile([128, 8, 128], F32, tag="s",
                                           bufs=s_bufs, name="st")
                            for j8 in range(8):
                                jt = 8 * half + j8
                                nc.tensor.matmul(
                                    sp[:, j8, :],
                                    lhsT=qk8[base:base + 32, ftk, :,
                                             ts(jt, 128)],
                                    rhs=qk8[base:base + 32, ftq, :,
                                            it * 512 + 128 * qs:
                                            it * 512 + 128 * (qs + 1)],
                                    start=True, stop=True,
                                    perf_mode=DR, tile_position=(base, 0))
                            pexp = pxpool.tile([128, 8, 128], BF16, tag="pxt",
                                               bufs=2, name="pxt")
                            nc.scalar.activation(
                                pexp, sp[:],
                                mybir.ActivationFunctionType.Exp,
                                scale=0.125 / (WS * WS))
                            if half == 0:
                                # partner head's lagged attn@v for this qs
                                if pb["yu"] is None:
                                    pb["yu"] = psum.tile(
                                        [128, 4, D + 1], F32, tag="yu",
                                        bufs=yu_bufs, name="yu")
                                lag_attnv(pb, qs)
                                rcpp = npool.tile([128, 1], F32, tag="rcp",
                                                  name="rcpp")
                                nc.vector.reciprocal(
                                    rcpp, pb["yu"][:, qs, D:D + 1])
                                nc.vector.tensor_scalar_mul(
                                    ynp[:, qs, 0, :],
                                    pb["yu"][:, qs, 0:D], rcpp)
                            for j8 in range(8):
                                jt = 8 * half + j8
                                nc.tensor.matmul(
                                    yu[:, qs, :],
                                    lhsT=pexp[:, j8, :],
                                    rhs=v_sb[:, jt, h, :],
                                    start=(jt == 0), stop=(jt == JT - 1))
                        rcp = npool.tile([128, 1], F32, tag="rcp")
                        nc.vector.reciprocal(rcp, yu[:, qs, D:D + 1])
                        nc.vector.tensor_scalar_mul(
                            ynp[:, qs, 1, :], yu[:, qs, 0:D], rcp)
                        yt2 = psum.tile([128, 128], F32, tag="pv",
                                        bufs=1, name="yt2t")
                        nc.tensor.transpose(
                            yt2, ynp[:, qs, :, :].rearrange("p a b -> p (a b)"),
                            ident_sb[:, :])
                        tt = 4 * it + qs
                        nc.vector.tensor_copy(
                            y2_sb[:, hp, ts(tt, 128)], yt2)
                        for on in range(DIM // 512):
                            if (tt, on) not in proj_done:
                                proj_tile(tt, on, tail=(qs == 3))

                # ---- phase A: minimal startup ----
                # all startup tiles split into 256-token halves: the rope
                # chains start as soon as the first x8/trig half lands
                qk_tile(0, 0, 0, borrow="s", terms=startup_terms,
                        tk0=0, tkn=256)
                qk_tile(0, 1, 0, borrow="s", terms=startup_terms,
                        tk0=0, tkn=256)
                qk_tile(1, 0, 0, borrow="pv", terms=startup_terms,
                        tk0=0, tkn=256)
                qk_tile(1, 1, 0, borrow="pqk", terms=startup_terms,
                        tk0=0, tkn=256)
                qk_tile(0, 0, 0, borrow="s", terms=startup_terms,
                        tk0=256, tkn=256)
                qk_tile(0, 1, 0, borrow="s", terms=startup_terms,
                        tk0=256, tkn=256)
                qk_tile(1, 0, 0, borrow="pv", terms=startup_terms,
                        tk0=256, tkn=256)
                qk_tile(1, 1, 0, borrow="pqk", terms=startup_terms,
                        tk0=256, tkn=256)
                for _ in range(3):
                    v_tile(v_done[0]); v_done[0] += 1

                # A-it0 with a lag-2 ramp: blocks 0-1 lag nothing (the
                # v tiles own the PE), block2 lags blk0 late, block3 lags
                # blk1, block4 catches up with blk2+blk3, then steady lag-1
                blks = {}

                def ab(h, it, **kw):
                    att_block(h, it, **kw)
                    blks[(h, it)] = prev_blk[0]

                ab(0, 0, lags=())
                ab(1, 0, lags=())
                ab(2, 0, lags=(blks[(0, 0)],), lag_jcs=(4, 5, 6, 7))
                ab(3, 0, lags=(blks[(1, 0)],))
                ab(0, 1, lags=(blks[(2, 0)], blks[(3, 0)]))
                for h in range(1, 4):
                    ab(h, 1)
                for h in range(4, 8):
                    ab(h, 0)
                for h in range(4):
                    ab(h, 2)
                for h in range(4, 8):
                    ab(h, 1)
                for h in range(4):
                    ab(h, 3)
                for h in range(4, 8):
                    ab(h, 2)
                # B-it3: h6 before h7; h7 last with the per-qs tail dribble
                ab(4, 3)
                ab(5, 3)
                ab(6, 3)
                att_block_tail(7, 3)

                while PENDING_PE:
                    PENDING_PE.pop(0)()

                if debug_y2:
                    nc.sync.dma_start(
                        y2dbg_d[:, :],
                        y2_sb.rearrange("p a t -> p (a t)"))

                for tt in range(TT):
                    for on in range(DIM // 512):
                        if (tt, on) not in proj_done:
                            proj_tile(tt, on, wide=True)

    nc.finalize()
    return nc


_NC_CACHE = None


def _get_nc():
    global _NC_CACHE
    if _NC_CACHE is None:
        _NC_CACHE = build_nc()
    return _NC_CACHE


# ---------------------------------------------------------------- entry point

def kernel(x, freqs, W_qkv, W_proj, b_proj, _trace=False):
    x = np.asarray(x, dtype=np.float32)
    freqs = np.asarray(freqs, dtype=np.float32)
    W_qkv = np.asarray(W_qkv, dtype=np.float32)
    W_proj = np.asarray(W_proj, dtype=np.float32)
    b_proj = np.asarray(b_proj, dtype=np.float32)

    dm = _dmap()
    cosP, sinS = _cos_sin_tiles(freqs)

    def split8(a):
        hi = a.astype(ml_dtypes.float8_e4m3fn)
        lo = (a - hi.astype(np.float32)).astype(ml_dtypes.float8_e4m3fn)
        return hi, lo

    wqk8T, wqklT, wv8T, wvlT, wpT = {}, {}, {}, {}, {}
    for g in range(G):
        cols = []
        for ft in range(NFT):
            qk = ft % 2
            for u in range(2):
                for hq in range(4):
                    h = g * HG + 4 * (ft // 2) + hq
                    base = qk * DIM + h * D
                    cols.append(W_qkv[base + dm[:, u]])
        wqkT_full = np.ascontiguousarray(
            np.concatenate(cols, axis=0).T).astype(np.float32) * WS
        wqk8T[g], wqklT[g] = split8(wqkT_full)
        wvT_full = np.ascontiguousarray(
            W_qkv[2 * DIM + g * DV: 2 * DIM + (g + 1) * DV].T
        ).astype(np.float32) * WS
        wv8T[g], wvlT[g] = split8(wvT_full)
        wpT[g] = np.ascontiguousarray(
            W_proj[:, g * DV:(g + 1) * DV].T).astype(ml_dtypes.bfloat16)

    x8_b, xl_b = {}, {}
    for b in range(B):
        x8_b[b], xl_b[b] = split8(np.ascontiguousarray(x[b].T))

    trig = np.concatenate(
        [cosP.reshape(128, 2 * T), sinS.reshape(128, 2 * T)],
        axis=1)  # [128, 2*2*T] bf16: cos block then sin block
    wv2 = {g: np.concatenate([wv8T[g], wvlT[g]], axis=1) for g in range(G)}
    ident = np.eye(128, dtype=np.float32)
    in_maps = []
    for core in range(N_CORES):
        b, g = core // G, core % G
        in_maps.append({
            "x8": x8_b[b],
            "xl8": xl_b[b],
            "wqk8": wqk8T[g],
            "wqkl8": wqklT[g],
            "wv28": wv2[g],
            "wpT": wpT[g],
            "trig": trig,
            "ident": ident,
        })

    from concourse import bass_utils

    nc = _get_nc()
    res = bass_utils.run_bass_kernel_spmd(
        nc, in_maps, core_ids=list(range(N_CORES)), trace=_trace)

    out = np.zeros((B, T, DIM), dtype=np.float32)
    for core in range(N_CORES):
        b = core // G
        out[b] += res.results[core]["out_part"]
    out += b_proj
    if _trace:
        return out, res
    return out
